# revision 1
# baseline (speedup 1.0000x reference)
"""Trainium2 Bass kernel for a 2-path threshold MoE router (BranchRoute).

Semantics (must match the reference):
    score = sigmoid(x @ W + b)                  # [N, 2]
    mask_p = score[:, p] >= 0.5   (== logit >= 0)
    rank_p = cumsum(mask_p) - 1                 # global pack order
    x_p[rank_p[i]] = x[i] for masked i, zero-padded to N rows
    combined[i] = (mask_0[i] + mask_1[i]) * x[i]
    returns (x0, x1, combined)

Strategy: data-parallel over tokens on 8 NeuronCores. Each core:
  - reads its 4096-token shard of x once,
  - computes gate logits with fused DVE multiply+reduce ops
    (tensor_tensor_reduce) against a pre-broadcast copy of W,
  - builds per-128-token masks, in-tile ranks via a triangular matmul,
    and a running cross-tile base count via tiny PE matmuls,
  - writes `combined` with a ScalarE copy scaled by (m0+m1),
  - compacts routed rows into its local packed buffers with indirect
    (scatter) DMAs; out-of-path rows get a huge destination index and are
    dropped by the bounds check.
Host side: per-core packed buffers are concatenated (each shard's routed
rows occupy a contiguous global range, in shard order), which is the
unshard step for the global cumsum pack order.
"""

import numpy as np

N = 32768
D = 2048
P = 2
NCORES = 8
NLOC = N // NCORES  # 4096 tokens per core
TP = 128            # tokens per group (one SBUF partition block)
G = 2               # groups per macro tile
NMT = NLOC // (TP * G)  # macro tiles per core
BIG = 1.0e6         # offset pushed onto non-routed destinations (-> OOB drop)

_nc = None
_nc_variant = None


def _build_nc(variant="full", repeat=1):
    import concourse.bass as bass
    import concourse.bacc as bacc
    import concourse.tile as tile
    from concourse import mybir
    from contextlib import ExitStack

    f32 = mybir.dt.float32
    i32 = mybir.dt.int32

    # Bacc (not raw Bass): its finalize() runs the lowering passes that
    # split multi-semaphore waits into standalone event-semaphore waits —
    # TRN2 instructions can carry at most one sync wait.
    nc = bacc.Bacc()

    x_h = nc.dram_tensor("x", [NLOC, D], f32, kind="ExternalInput")
    wb_h = nc.dram_tensor("wb", [TP, P, D], f32, kind="ExternalInput")
    bb_h = nc.dram_tensor("bb", [TP, P], f32, kind="ExternalInput")  # -b bcast
    tri_h = nc.dram_tensor("tribig", [TP, TP], f32, kind="ExternalInput")
    onescol_h = nc.dram_tensor("onescol", [TP, 1], f32, kind="ExternalInput")
    onesk1_h = nc.dram_tensor("onesk1", [1, TP], f32, kind="ExternalInput")
    zrow_h = nc.dram_tensor("zrow", [1, P], f32, kind="ExternalInput")

    x0_h = nc.dram_tensor("x0", [NLOC, D], f32, kind="ExternalOutput")
    x1_h = nc.dram_tensor("x1", [NLOC, D], f32, kind="ExternalOutput")
    comb_h = nc.dram_tensor("comb", [NLOC, D], f32, kind="ExternalOutput")
    counts_h = nc.dram_tensor("counts", [1, P], i32, kind="ExternalOutput")
    xp_out = [x0_h, x1_h]

    with ExitStack() as ctx:
        tc = ctx.enter_context(tile.TileContext(nc))
        singles = ctx.enter_context(tc.tile_pool(name="singles", bufs=1))
        xp = ctx.enter_context(tc.tile_pool(name="xp", bufs=5))
        zp = ctx.enter_context(tc.tile_pool(name="zp", bufs=3))
        cp = ctx.enter_context(tc.tile_pool(name="cp", bufs=3))
        sm = ctx.enter_context(tc.tile_pool(name="sm", bufs=6))
        bp = ctx.enter_context(tc.tile_pool(name="bp", bufs=4))
        ps = ctx.enter_context(tc.tile_pool(name="ps", bufs=2, space="PSUM"))
        pc = ctx.enter_context(tc.tile_pool(name="pc", bufs=2, space="PSUM"))

        wb_sb = singles.tile([TP, P, D], f32)
        nc.sync.dma_start(out=wb_sb[:], in_=wb_h[:])
        bb_sb = singles.tile([TP, P], f32)
        nc.sync.dma_start(out=bb_sb[:], in_=bb_h[:])
        tri_sb = singles.tile([TP, TP], f32)
        nc.sync.dma_start(out=tri_sb[:], in_=tri_h[:])
        onescol_sb = singles.tile([TP, 1], f32)
        nc.sync.dma_start(out=onescol_sb[:], in_=onescol_h[:])
        onesk1_sb = singles.tile([1, TP], f32)
        nc.sync.dma_start(out=onesk1_sb[:], in_=onesk1_h[:])
        base0_sb = singles.tile([1, P], f32)
        nc.sync.dma_start(out=base0_sb[:], in_=zrow_h[:])

        base_cur = base0_sb
        bc_reg = nc.gpsimd.to_reg(NLOC - 1)

        # Dummy matmuls so the PE consumes each constant's DMA-completion
        # semaphore once, up front. Walrus can encode only one sync-wait on
        # a Matmult (it lands on the LDWEIGHTS slot), so the real per-tile
        # matmuls must not also need to wait on these loads.
        warm_ps = ps.tile([TP, P], f32, tag="warm")
        nc.tensor.matmul(
            out=warm_ps[:],
            lhsT=tri_sb[:],
            rhs=tri_sb[:, 0:P],
            start=True,
            stop=True,
            skip_group_check=True,
        )
        warm_ps2 = pc.tile([1, P], f32, tag="warm2")
        nc.tensor.matmul(
            out=warm_ps2[:],
            lhsT=onescol_sb[:],
            rhs=tri_sb[:, 0:P],
            start=True,
            stop=True,
            skip_group_check=True,
        )
        warm_ps3 = ps.tile([TP, P], f32, tag="warm")
        nc.tensor.matmul(
            out=warm_ps3[:],
            lhsT=onesk1_sb[:],
            rhs=onesk1_sb[:, 0:P],
            start=True,
            stop=True,
            skip_group_check=True,
        )

        for rep in range(repeat):
          base_cur = base0_sb
          for mt in range(NMT):
            row0 = mt * TP * G
            x_t = xp.tile([TP, G, D], f32, tag="x_t")
            load_eng = nc.sync
            store_eng = nc.sync
            load_eng.dma_start(
                out=x_t[:],
                in_=x_h[row0 : row0 + TP * G, :].rearrange(
                    "(a p) d -> p a d", p=TP
                ),
            )
            comb_t = cp.tile([TP, G, D], f32, tag="comb_t")
            dest_mt = sm.tile([TP, P, G], i32, tag="dest_mt")

            for a in range(G):
                xg = x_t[:, a, :]

                # gate logits: s[:, p] = sum_d x[tok, d] * W[d, p]
                sg = sm.tile([TP, P], f32, tag="sg")
                for p in range(P):
                    z_t = zp.tile([TP, D], f32, tag="z_t")
                    nc.vector.scalar_tensor_tensor(
                        out=z_t[:],
                        in0=xg,
                        scalar=1.0,
                        in1=wb_sb[:, p, :],
                        op0=mybir.AluOpType.mult,
                        op1=mybir.AluOpType.mult,
                        accum_out=sg[:, p : p + 1],
                    )

                # masks: m = (s + b) >= 0  <=>  s >= -b   (bb_sb holds -b)
                m_g = sm.tile([TP, P], f32, tag="m_g")
                nc.vector.tensor_tensor(
                    out=m_g[:],
                    in0=sg[:],
                    in1=bb_sb[:],
                    op=mybir.AluOpType.is_ge,
                )

                # combined = (m0 + m1) * x
                msum = sm.tile([TP, 1], f32, tag="msum")
                nc.vector.tensor_add(
                    out=msum[:], in0=m_g[:, 0:1], in1=m_g[:, 1:2]
                )
                nc.scalar.activation(
                    out=comb_t[:, a, :],
                    in_=xg,
                    func=mybir.ActivationFunctionType.Copy,
                    scale=msum[:, 0:1],
                )

                # per-group counts and running base
                cnt_ps = pc.tile([1, P], f32, tag="cnt")
                nc.tensor.matmul(
                    out=cnt_ps[:],
                    lhsT=onescol_sb[:],
                    rhs=m_g[:],
                    start=True,
                    stop=True,
                )
                basebig = bp.tile([1, P], f32, tag="basebig")
                nc.vector.tensor_scalar_add(
                    out=basebig[:], in0=base_cur[:], scalar1=BIG - 1.0
                )
                base_next = bp.tile([1, P], f32, tag="base")
                nc.vector.tensor_add(
                    out=base_next[:], in0=base_cur[:], in1=cnt_ps[:]
                )
                base_cur = base_next

                # dest = (base - 1 + BIG) + (TRI - BIG*I) @ m
                #   masked row i  -> base + (# masked j<=i) - 1   (its rank)
                #   unmasked row  -> ~BIG                          (dropped)
                dest_ps = ps.tile([TP, P], f32, tag="dest")
                nc.tensor.matmul(
                    out=dest_ps[:],
                    lhsT=onesk1_sb[:],
                    rhs=basebig[:],
                    start=True,
                    stop=False,
                    skip_group_check=True,
                )
                nc.tensor.matmul(
                    out=dest_ps[:],
                    lhsT=tri_sb[:],
                    rhs=m_g[:],
                    start=False,
                    stop=True,
                    skip_group_check=True,
                )
                nc.vector.tensor_copy(
                    out=dest_mt[:, :, a], in_=dest_ps[:]
                )

                if variant != "noscatter":
                    for p in range(P):
                        nc.gpsimd.indirect_dma_start(
                            out=xp_out[p][:],
                            out_offset=bass.IndirectOffsetOnAxis(
                                ap=dest_mt[:, p, a : a + 1], axis=0
                            ),
                            in_=xg,
                            in_offset=None,
                            bounds_check=bc_reg,
                            oob_is_err=False,
                        )

            # alternate rings so loads and stores split evenly across the
            # two HWDGE FIFOs (SP and ACT)
            store_eng.dma_start(
                out=comb_h[row0 : row0 + TP * G, :].rearrange(
                    "(a p) d -> p a d", p=TP
                ),
                in_=comb_t[:],
            )

        counts_i = sm.tile([1, P], i32, tag="counts_i")
        nc.vector.tensor_copy(out=counts_i[:], in_=base_cur[:])
        nc.sync.dma_start(out=counts_h[:], in_=counts_i[:])

    nc.finalize()
    return nc


def _get_nc():
    global _nc, _nc_variant
    import os

    variant = os.environ.get("BR_VARIANT", "full")
    repeat = int(os.environ.get("BR_REPEAT", "1"))
    key = (variant, repeat)
    if _nc is None or _nc_variant != key:
        _nc = _build_nc(variant, repeat)
        _nc_variant = key
    return _nc


def _make_const_inputs(W32, b32):
    wb = np.ascontiguousarray(
        np.broadcast_to(W32.T[None, :, :], (TP, P, D)), dtype=np.float32
    )
    bb = np.ascontiguousarray(
        np.broadcast_to(-b32[None, :], (TP, P)), dtype=np.float32
    )
    j = np.arange(TP)
    tri = (j[:, None] <= j[None, :]).astype(np.float32)
    tri[j, j] -= np.float32(BIG)
    consts = {
        "wb": wb,
        "bb": bb,
        "tribig": np.ascontiguousarray(tri),
        "onescol": np.ones((TP, 1), np.float32),
        "onesk1": np.ones((1, TP), np.float32),
        "zrow": np.zeros((1, P), np.float32),
    }
    return consts


def run_on_cores(x, W, b, trace=False):
    """Compile (cached) + run the SPMD kernel; returns (results, bass_results)."""
    from concourse.bass_utils import run_bass_kernel_spmd

    x = np.ascontiguousarray(np.asarray(x, dtype=np.float32))
    W32 = np.ascontiguousarray(np.asarray(W, dtype=np.float32))
    b32 = np.ascontiguousarray(np.asarray(b, dtype=np.float32))
    assert x.shape == (N, D) and W32.shape == (D, P) and b32.shape == (P,)

    nc = _get_nc()
    consts = _make_const_inputs(W32, b32)
    in_maps = [
        {"x": x[c * NLOC : (c + 1) * NLOC], **consts} for c in range(NCORES)
    ]
    out = run_bass_kernel_spmd(nc, in_maps, list(range(NCORES)), trace=trace)
    return out.results, out


def kernel(x, W, b):
    results, _ = run_on_cores(x, W, b)

    x0 = np.zeros((N, D), np.float32)
    x1 = np.zeros((N, D), np.float32)
    comb = np.empty((N, D), np.float32)
    p0 = p1 = 0
    for c in range(NCORES):
        r = results[c]
        k0 = int(r["counts"][0, 0])
        k1 = int(r["counts"][0, 1])
        x0[p0 : p0 + k0] = r["x0"][:k0]
        x1[p1 : p1 + k1] = r["x1"][:k1]
        comb[c * NLOC : (c + 1) * NLOC] = r["comb"]
        p0 += k0
        p1 += k1
    return x0, x1, comb

